# revision 14
# baseline (speedup 1.0000x reference)
"""Multi-head attention (N=4, L=2048, D=512, H=8) on 8 Trainium2 NeuronCores.

Sharding: 8 cores = 4 batches x 2 query-block parities. Core (n, p) handles
batch n and query blocks g = 2m+p (m=0..7, blocks of 128 rows), so every core
has the identical causal workload shape: local block slot m only needs key
tiles jt < min(2m+2, T*), where T* = ceil(max_len/128) is compiled in from
the actual padding_mask. This skips the strictly-upper-triangular half of the
score/AV/exp work that a contiguous query split computes and then masks away,
and keeps all 8 cores load-balanced.

Masking:
  - padding: folded into V — V rows (including the softmax-denominator ones
    column) are zeroed for padded keys, so padded keys contribute 0 to both
    numerator and denominator. No per-score padding multiply.
  - causal: only the diagonal 128x128 block of each (slot, key-tile) pair
    needs a mask; it is the same [128, 2, 128] tile for every slot:
    slot parity == core parity -> lower-triangular, below -> all-ones
    (interior), above -> all-zeros (the wrong-parity tile the shared SPMD
    schedule computes but this core doesn't need).

Precision: Q/K projections run on fp8 e4m3 operands with DoubleRow perf mode
(two 128-deep k-tiles per pass -> half the PE cycles; W and b pre-scaled x32
on the host so e4m3 stays in its normal range, the exp scale divides the
x1024 back out). Everything downstream (scores, softmax, AV, output
projection) is fp16 with fp32 PSUM accumulation. Measured absmax relative
error vs the fp32 reference: ~1.1e-2 (tolerance 2e-2); fp8 anywhere in the
V path fails because short causal rows average only a few V entries.

Host staging (layout/dtype only; all arithmetic happens on device): x and W
arrive pre-swizzled to the SBUF layouts ([p, jb, k, 512] / [p, k, d_out]) so
every DMA is a 2-4KB contiguous run per partition.

Per-core pipeline:
  1. QT[d,i], KT[d,j] (f16, fp8-DR matmuls + DVE bias add) and V[j,d]
     (natural, f16, heads interleaved with a ones column per head; +bv then
     x pad[j]).
  2. Per (head, jt < T*): ST[j,i] = K Q^T on the PE for i in [128*(jt//2),
     LQ) only (even heads use PE tile (0,0), odd (64,0)); P = exp(ST*scale)
     on ACT straight from PSUM into pair-slot jt%2 of a [128, 2, LQ] tile;
     one DVE multiply per pair applies the diagonal mask to both slots;
     VT[65,i] += Vaug^T P with the ones column accumulating the softmax
     denominator in PSUM row 64.
  3. Per head: 1/den via one DVE reciprocal of PSUM row 64, broadcast to 64
     partitions by a k=1 ones matmul; one multiply per pair into VT (f16).
  4. out[i,:] = VTn^T @ WoT (+bo), DMA'd out row-wise (parity-permuted rows;
     the host gather undoes the permutation).

Performance notes: the chip's HAM activity throttle halves the clock ~50% of
the attention phase whenever PE+ACT(+DVE) run concurrently (pure-PE streams
never throttle; measured), so the kernel is paced by the throttled PE stream
and the ACT exp volume. Matmul PSUM outputs must stay within one 2KB bank,
and start=True lazily zeroes the whole bank (so only a bank's first writer
may set it). Typical HW exec ~195-205us (was 269us for the naive
query-halves split).
"""

import numpy as np
import ml_dtypes

import concourse.bass as bass
import concourse.tile as tile
from concourse import bacc, mybir
from concourse.bass_utils import run_bass_kernel_spmd

F32 = mybir.dt.float32
F16 = mybir.dt.float16
F8 = mybir.dt.float8e4
DR = mybir.MatmulPerfMode.DoubleRow

N, L, D, H = 4, 2048, 512, 8
DK = D // H          # 64
NCORES = 8
LQ = L // 2          # queries per core
P = 128
DC = D // P          # 4 d-chunks
NJT = L // P         # 16 key tiles
NIT = LQ // P        # 8 query tiles per core
WSCALE = 32.0        # host pre-scale on Wq/Wk/bq/bk for e4m3 range


def tstar_from_padding(padding_mask) -> int:
    lengths = np.asarray(padding_mask).astype(np.int64).sum(axis=1)
    return max(8, int(np.ceil(lengths.max() / P)))


def build_nc(tstar=NJT):
    tstar = min(int(tstar), NJT)
    nc = bacc.Bacc("TRN2", target_bir_lowering=False, debug=False,
                   num_devices=NCORES)

    # activations/weights come in pre-swizzled to the SBUF layout so every
    # DMA is a 2-4KB contiguous run per partition
    xq8 = nc.dram_tensor("xq8", [P, LQ // 512, DC, 512], F8,
                         kind="ExternalInput").ap()
    xk8 = nc.dram_tensor("xk8", [P, L // 512, DC, 512], F8,
                         kind="ExternalInput").ap()
    xvT = nc.dram_tensor("xvT", [P, L // 512, DC, 512], F16,
                         kind="ExternalInput").ap()
    wq8 = nc.dram_tensor("wq8", [P, DC, D], F8, kind="ExternalInput").ap()
    wk8 = nc.dram_tensor("wk8", [P, DC, D], F8, kind="ExternalInput").ap()
    wvT = nc.dram_tensor("wvT", [P, DC, D], F16, kind="ExternalInput").ap()
    woT = nc.dram_tensor("woT", [P, DC, D], F16, kind="ExternalInput").ap()
    bq = nc.dram_tensor("bq", [D], F32, kind="ExternalInput").ap()
    bk = nc.dram_tensor("bk", [D], F32, kind="ExternalInput").ap()
    bv = nc.dram_tensor("bv", [D], F32, kind="ExternalInput").ap()
    bo = nc.dram_tensor("bo", [D], F32, kind="ExternalInput").ap()
    mask128 = nc.dram_tensor("mask128", [P, 4, P], F16, kind="ExternalInput").ap()
    pad = nc.dram_tensor("pad", [L], F32, kind="ExternalInput").ap()
    out = nc.dram_tensor("out", [LQ, D], F32, kind="ExternalOutput").ap()

    with tile.TileContext(nc) as tc, nc.allow_low_precision(
            reason="fp8/f16 matmul operands; accumulation stays f32"):
        build_kernel(tc, xq8, xk8, xvT, wq8, wk8, wvT, woT,
                     bq, bk, bv, bo, mask128, pad, out, tstar)
    nc.compile()
    return nc


def build_kernel(tc, xq8, xk8, xvT, wq8, wk8, wvT, woT,
                 bq, bk, bv, bo, mask128, pad, out, tstar):
    nc = tc.nc
    Exp = mybir.ActivationFunctionType.Exp

    def chunks(start):
        # i-column ranges [c0, c1) covering [start, LQ), cut on the 512 grid
        # (PSUM bank boundaries)
        cs = []
        c0 = start
        while c0 < LQ:
            c1 = min((c0 // 512 + 1) * 512, LQ)
            cs.append((c0, c1))
            c0 = c1
        return cs

    njt = min(NJT, tstar)

    def last_jt_for(c0):
        # last key tile whose column range includes chunk starting at c0
        return min(njt - 1, 2 * (c0 // P) + 1)

    with (
        tc.tile_pool(name="persist", bufs=1) as persist,
        tc.tile_pool(name="bigpersist", bufs=1) as bigpersist,
    ):
        # ---- persistent tiles --------------------------------------------
        qt_sb = bigpersist.tile([P, DC, LQ], F16, tag="qt")
        kt_sb = bigpersist.tile([P, DC, L], F16, tag="kt")
        # V natural [j, d], fp16, heads interleaved with a ones column after
        # each head's 64 dims: [j-tile, head, 65]
        v_sb = bigpersist.tile([P, NJT, H, DK + 1], F16, tag="v")
        nc.vector.memset(v_sb[:, :, :, DK:DK + 1], 1.0)
        # wo/bo are only needed late; their DMAs are emitted after the
        # projections so the Q/K/V-critical DMAs win the queue.
        wo_sb = persist.tile([P, DC, D], F16, tag="wo")
        bo_bc = persist.tile([P, D], F32, tag="bo")
        mask_sb = persist.tile([P, 4, P], F16, tag="mask128")
        pad_sb = persist.tile([P, NJT], F32, tag="pad")

        # ---- projections --------------------------------------------------
        with (
            tc.tile_pool(name="wproj", bufs=1) as wproj,
            tc.tile_pool(name="xstage", bufs=4) as xstage,
            tc.tile_pool(name="projps", bufs=4, space="PSUM") as projps,
        ):
            wq_sb = wproj.tile([P, DC, D], F8, tag="wq")
            nc.sync.dma_start(out=wq_sb, in_=wq8)
            bq_col = wproj.tile([P, DC], F32, tag="bqc")
            nc.sync.dma_start(out=bq_col, in_=bq.rearrange("(c p) -> p c", p=P))
            wk_sb = wproj.tile([P, DC, D], F8, tag="wk")
            wv_sb = wproj.tile([P, DC, D], F16, tag="wv")
            bk_col = wproj.tile([P, DC], F32, tag="bkc")
            bv_bc = wproj.tile([P, D], F32, tag="bvbc")

            # Q projection first (all scores need it), then K and V
            # interleaved per j-block so attention can start early.
            # Q/K matmuls are fp8 DoubleRow: contraction pairs of 128-deep
            # k-tiles, so 2 matmuls instead of 4 per (c, block), at half the
            # cycles per output column.
            def stage(xT, jb, dtype, tag, cols=512):
                xt = xstage.tile([P, DC, 512], dtype, tag=tag, name="xt")
                nc.sync.dma_start(out=xt[:, :, 0:cols],
                                  in_=xT[:, jb, :, 0:cols])
                return xt

            def qk_proj(w_sb, b_col, out_sb, xt, jb, cols=512):
                for c in range(DC):
                    ps = projps.tile([P, 512], F32, tag="projps")
                    for ch0 in range(0, cols, 256):
                        chw = min(256, cols - ch0)
                        for u in range(2):
                            nc.tensor.matmul(
                                ps[:, ch0:ch0 + chw],
                                lhsT=w_sb[:, 2 * u:2 * u + 2,
                                          c * P:(c + 1) * P],
                                rhs=xt[:, 2 * u:2 * u + 2, ch0:ch0 + chw],
                                start=(u == 0), stop=(u == 1),
                                perf_mode=DR, skip_group_check=True)
                    nc.scalar.activation(
                        out=out_sb[:, c, jb * 512:jb * 512 + cols],
                        in_=ps[:, 0:cols],
                        func=mybir.ActivationFunctionType.Identity,
                        bias=b_col[:, c:c + 1])

            def v_proj(xt, jb):
                for jtl in range(4):
                    jt = jb * 4 + jtl
                    if jt >= njt:
                        continue
                    ps = projps.tile([P, D], F32, tag="projpsv")
                    for k in range(DC):
                        nc.tensor.matmul(
                            ps, lhsT=xt[:, k, jtl * P:(jtl + 1) * P],
                            rhs=wv_sb[:, k, :],
                            start=(k == 0), stop=(k == DC - 1))
                    nc.vector.tensor_add(
                        out=v_sb[:, jt, :, 0:DK],
                        in0=ps.rearrange("p (h d) -> p h d", h=H),
                        in1=bv_bc.rearrange("p (h d) -> p h d", h=H))
                    # fold padding: zero V rows (incl. ones column) for
                    # padded keys -> they drop out of numerator+denominator
                    nc.vector.tensor_scalar_mul(
                        out=v_sb[:, jt, :, :], in0=v_sb[:, jt, :, :],
                        scalar1=pad_sb[:, jt:jt + 1])

            # DMA issue order follows need order so transfers (which
            # serialize on the queue) land just ahead of their consumers.
            ktoks = njt * P          # only the first T* key tiles are live
            kcols = [min(512, max(0, ktoks - jb * 512))
                     for jb in range(L // 512)]
            xq_t = [stage(xq8, 0, F8, "xstage8")]
            xq_t.append(stage(xq8, 1, F8, "xstage8"))
            nc.sync.dma_start(out=wk_sb, in_=wk8)
            nc.sync.dma_start(out=bk_col, in_=bk.rearrange("(c p) -> p c", p=P))
            xk_t = [stage(xk8, 0, F8, "xstage8", cols=kcols[0])]
            nc.sync.dma_start(out=wv_sb, in_=wvT)
            nc.sync.dma_start(
                out=bv_bc,
                in_=bass.AP(tensor=bv.tensor, offset=bv.offset,
                            ap=[[0, P], [1, D]]))
            nc.sync.dma_start(out=pad_sb, in_=pad.rearrange("(t p) -> p t", p=P))
            xv_t = [stage(xvT, 0, F16, "xstage")]
            qk_proj(wq_sb, bq_col, qt_sb, xq_t[0], 0)
            qk_proj(wq_sb, bq_col, qt_sb, xq_t[1], 1)
            # warm up the GPSIMD partition-broadcast ucode (one-time ~5us
            # library load) while the PE is busy with projections
            rwarm = wproj.tile([DK, 16], F32, tag="rwarm")
            nc.gpsimd.partition_broadcast(rwarm, pad_sb[0:1, 0:NJT],
                                          channels=DK)
            for jb in range(L // 512):
                if jb + 1 < L // 512:
                    if kcols[jb + 1] > 0:
                        xk_t.append(stage(xk8, jb + 1, F8, "xstage8",
                                          cols=kcols[jb + 1]))
                    xv_t.append(stage(xvT, jb + 1, F16, "xstage"))
                if kcols[jb] > 0:
                    qk_proj(wk_sb, bk_col, kt_sb, xk_t[jb], jb,
                            cols=kcols[jb])
                v_proj(xv_t[jb], jb)
            nc.sync.dma_start(out=mask_sb, in_=mask128)
            nc.sync.dma_start(out=wo_sb, in_=woT)
            nc.sync.dma_start(
                out=bo_bc,
                in_=bass.AP(tensor=bo.tensor, offset=bo.offset,
                            ap=[[0, P], [1, D]]))

        # ---- attention ----------------------------------------------------
        # Work unit (job) = (head-pair, query-chunk c of 256, key-pair t).
        # The two heads of a pair occupy PE row groups 0-63 / 64-127, so
        # their score matmuls execute CONCURRENTLY on the (throttled) PE.
        # One ACTIVATE per job exps all 4 slots (2 heads x 2 key slots)
        # straight from a [P, 4, 256] PSUM tile; AV accumulates both heads
        # into one shared-bank [65, 2, 256] tile whose row 64 collects the
        # softmax denominators via the ones column of V. A lag-2 software
        # pipeline keeps the PE stream dense.
        with (
            tc.tile_pool(name="stps", bufs=3, space="PSUM") as stps,
            tc.tile_pool(name="vtps", bufs=2, space="PSUM") as vtps,
            tc.tile_pool(name="ppool", bufs=5) as ppool,
            tc.tile_pool(name="rpool", bufs=4) as rpool,
            tc.tile_pool(name="obuf", bufs=3) as obuf,
        ):
            vtn_sb = bigpersist.tile([P, DC, LQ], F16, tag="vtn")
            escale = 1.0 / (np.sqrt(DK) * WSCALE * WSCALE)
            W = 256

            jobs = []
            for c in range(LQ // W):
                for hp in range(H // 2):
                    pairs = [t for t in range(2 * c + 2) if 2 * t < njt]
                    for i, t in enumerate(pairs):
                        jobs.append((hp, c, t, i == 0, i == len(pairs) - 1))

            vt_of = {}
            norm_deferred = []

            def issue_front(job):
                hp, c, t, first, last = job
                ns = 2 if 2 * t + 1 < njt else 1
                a = max(P * t - W * c, 0)         # 0 or 128 within the chunk
                cols = (W * c + a, W * (c + 1))
                # slot layout is he-major (he*2 + s) so the two heads'
                # CONCURRENT row-tiled score matmuls write different PSUM
                # banks (bank = he); within a bank the s=0 MM is the first
                # writer (start=True zeroes the bank), s=1 overwrites its
                # own still-unwritten region with start=False.
                st = stps.tile([P, 4, W], F32, tag="st")
                for s in range(ns):
                    jt = 2 * t + s
                    for he in range(2):
                        ho = he * DK
                        nc.tensor.matmul(
                            st[:, 2 * he + s, a:],
                            lhsT=kt_sb[ho:ho + DK, hp, jt * P:(jt + 1) * P],
                            rhs=qt_sb[ho:ho + DK, hp, cols[0]:cols[1]],
                            start=(s == 0), stop=(s == ns - 1),
                            skip_group_check=True)
                pp = ppool.tile([P, 4, W], F16, tag="pp")
                if ns == 2:
                    nc.scalar.activation(out=pp[:, :, a:], in_=st[:, :, a:],
                                         func=Exp, scale=escale)
                else:
                    for he in range(2):
                        nc.scalar.activation(
                            out=pp[:, 2 * he:2 * he + 1, a:],
                            in_=st[:, 2 * he:2 * he + 1, a:],
                            func=Exp, scale=escale)
                if t >= 2 * c:
                    # this job's 128-wide region is the diagonal band
                    if ns == 2:
                        nc.vector.tensor_mul(pp[:, :, a:a + P],
                                             pp[:, :, a:a + P],
                                             mask_sb)
                    else:
                        for he in range(2):
                            nc.vector.tensor_mul(
                                pp[:, 2 * he:2 * he + 1, a:a + P],
                                pp[:, 2 * he:2 * he + 1, a:a + P],
                                mask_sb[:, 2 * he:2 * he + 1, :])
                return (job, pp, a, ns)

            def issue_back(ctx):
                job, pp, a, ns = ctx
                hp, c, t, first, last = job
                if first:
                    vt_of[(hp, c)] = vtps.tile([DK + 1, 2, W], F32,
                                               tag="vt", name="vt")
                vt = vt_of[(hp, c)]
                for s in range(ns):
                    jt = 2 * t + s
                    for he in range(2):
                        nc.tensor.matmul(
                            vt[:, he, a:],
                            lhsT=v_sb[:, jt, 2 * hp + he, :],
                            rhs=pp[:, 2 * he + s, a:],
                            start=(first and s == 0 and he == 0),
                            stop=(last and s == ns - 1),
                            skip_group_check=True)
                if last:
                    # both heads' denominators live on PSUM partition 64 as
                    # [1, 2, W]; one copy + one approx-reciprocal + one
                    # GPSIMD partition-broadcast serve both heads.
                    dent = rpool.tile([1, 2, W], F32, tag="dent")
                    nc.vector.tensor_copy(out=dent, in_=vt[DK:DK + 1, :, :])
                    rr = rpool.tile([1, 2, W], F32, tag="rr")
                    nc.vector.reciprocal_approx_fast(out=rr, in_=dent)
                    rbp = rpool.tile([DK, 2, W], F32, tag="rbp")
                    nc.gpsimd.partition_broadcast(rbp, rr, channels=DK)
                    # defer the multiplies one job so the DVE doesn't stall
                    # on the broadcast in front of the next job's mask
                    norm_deferred.append((hp, c, vt, rbp))

            def oproj_chunk(c):
                # query blocks 2c, 2c+1 are fully normalized; project and
                # store them now so the tail after the last exp is tiny
                for it in (2 * c, 2 * c + 1):
                    po = stps.tile([P, D], F32, tag="st", name="po")
                    for k in range(DC):
                        nc.tensor.matmul(
                            po, lhsT=vtn_sb[:, k, it * P:(it + 1) * P],
                            rhs=wo_sb[:, k, :], start=(k == 0),
                            stop=(k == DC - 1))
                    ob = obuf.tile([P, D], F32, tag="ob")
                    nc.vector.tensor_add(ob, po, bo_bc)
                    nc.sync.dma_start(out=out[it * P:(it + 1) * P, :],
                                      in_=ob)

            def flush_norms(keep=0):
                while len(norm_deferred) > keep:
                    hp, c, vt, rbp = norm_deferred.pop(0)
                    for he in range(2):
                        nc.vector.tensor_mul(
                            vtn_sb[he * DK:(he + 1) * DK, hp,
                                   W * c:W * (c + 1)],
                            vt[0:DK, he, :], rbp[:, he, :])
                    if hp == H // 2 - 1:
                        oproj_chunk(c)

            LAG = 2
            pending = []
            for job in jobs:
                pending.append(issue_front(job))
                if len(pending) > LAG:
                    issue_back(pending.pop(0))
                    flush_norms(keep=1)
            while pending:
                issue_back(pending.pop(0))
            flush_norms()



_NC_CACHE = {}


def _get_nc(tstar):
    if tstar not in _NC_CACHE:
        _NC_CACHE[tstar] = build_nc(tstar)
    return _NC_CACHE[tstar]


def _perm(p):
    # local column m*128+r  <->  global query (2m+p)*128+r
    return np.concatenate(
        [np.arange(P) + (2 * m + p) * P for m in range(NIT)])


def _mask128_const(p):
    # slot layout 2*s + head_parity: both heads of a PE-row-tiled pair get
    # the same per-key-slot mask
    tri = np.tril(np.ones((P, P), dtype=np.float16)).T  # [j, i]: j <= i
    m = np.empty((P, 4, P), dtype=np.float16)
    # he-major slot layout (he*2 + s); both heads get the same s-masks
    if p == 0:
        m[:, 0, :] = m[:, 2, :] = tri    # s=0 (jt = 2m): diagonal
        m[:, 1, :] = m[:, 3, :] = 0.0    # s=1 (jt = 2m+1 > g): empty
    else:
        m[:, 0, :] = m[:, 2, :] = 1.0    # s=0 (jt = 2m < g): interior
        m[:, 1, :] = m[:, 3, :] = tri    # s=1 (jt = 2m+1): diagonal
    return m


def make_in_maps(x_q, x_k, x_v, padding_mask, attention_mask,
                 Wq, bq, Wk, bk, Wv, bv, Wo, bo):
    f16, f32 = np.float16, np.float32
    E4 = ml_dtypes.float8_e4m3

    def w_swiz(W, dtype, scale=1.0):
        # [d_out, d_in] -> SBUF layout [p, k(d_in chunk), d_out]
        wt = np.asarray(scale * np.asarray(W, dtype=f32).T, dtype=dtype)
        return np.ascontiguousarray(wt.reshape(DC, P, D).transpose(1, 0, 2))

    def x_swiz(xT, dtype):
        # [d_in, M] -> [p, jb, c(d_in chunk), 512]
        M = xT.shape[1]
        x = np.asarray(xT, dtype=dtype)
        return np.ascontiguousarray(
            x.reshape(DC, P, M // 512, 512).transpose(1, 2, 0, 3))

    shared = {
        # Wq/Wk and bq/bk pre-scaled so e4m3 operands sit in normal range;
        # the exp scale divides the x1024 back out.
        "wq8": w_swiz(Wq, E4, WSCALE),
        "wk8": w_swiz(Wk, E4, WSCALE),
        "wvT": w_swiz(Wv, f16),
        "woT": w_swiz(Wo, f16),
        "bq": WSCALE * np.asarray(bq, dtype=f32),
        "bk": WSCALE * np.asarray(bk, dtype=f32),
        "bv": np.asarray(bv, dtype=f32), "bo": np.asarray(bo, dtype=f32),
    }
    masks = [_mask128_const(0), _mask128_const(1)]
    perms = [_perm(0), _perm(1)]
    xT = [np.asarray(x, dtype=f32).transpose(0, 2, 1)
          for x in (x_q, x_k, x_v)]
    in_maps = []
    for core in range(NCORES):
        n, p = divmod(core, 2)
        in_maps.append(dict(
            shared,
            xq8=x_swiz(xT[0][n][:, perms[p]], E4),
            xk8=x_swiz(xT[1][n], E4),
            xvT=x_swiz(np.asarray(xT[2][n], dtype=f16), f16),
            mask128=masks[p],
            pad=np.asarray(padding_mask[n], dtype=np.float32),
        ))
    return in_maps


def gather_out(results):
    full = np.empty((N, L, D), dtype=np.float32)
    perms = [_perm(0), _perm(1)]
    for core in range(NCORES):
        n, p = divmod(core, 2)
        full[n, perms[p], :] = results[core]["out"]
    return full


def kernel(x_q, x_k, x_v, padding_mask, attention_mask,
           Wq, bq, Wk, bk, Wv, bv, Wo, bo):
    nc = _get_nc(tstar_from_padding(padding_mask))
    in_maps = make_in_maps(x_q, x_k, x_v, padding_mask, attention_mask,
                           Wq, bq, Wk, bk, Wv, bv, Wo, bo)
    res = run_bass_kernel_spmd(nc, in_maps, core_ids=list(range(NCORES)))
    return gather_out(res.results)

